# revision 60
# baseline (speedup 1.0000x reference)
"""Multi-head attention (B=4, N=1024, C=1024, H=16) on 8 TRN2 NeuronCores.

Sharding: core c handles batch b = c//2 and query-row half g = c%2.
Data parallel over B; within a batch pair, K and V are tensor parallel
over heads.  All per-head state lives in VIRTUAL head order — the host
permutes W_q / W_k / W_p per core so that each core's own 8 heads are
virtual heads 0-7 — which makes the kernel program identical on every
core: attention rounds process own head-pairs first, so the K/V
exchange (three pipelined pair AllGathers through DRAM: KT fc0-1, V,
KT fc2-3) only has to land by rounds 4/5/6.  The peer's half is
fetched with indirect (index-tensor) DMAs into contiguous staging and
placed by VectorE — the per-core rank offset lives in the host-built
ridx input, because runtime registers (value_load / dynamic AP
offsets) crash this runtime, and strided indirect-DMA destinations
corrupt.  The host also orders each core's xT key-columns [own queries
| rest], so Q projects straight out of xT_s and no xqT load exists.

Schedule: rounds start as soon as k0,k1,q0 land (~30us); the V
projection (kc-outer across 8 PSUM accumulators carved from the
big+pv pools), remaining k/q chunks, the output-projection prefix
accumulations, and the woven PV matmuls all backfill TensorE inside
the ScalarE(exp)-bound rounds.  The first CC op only executes ~19us
after the ~35us start-skew barrier resolves, so the CC chain (not
compute) gates rounds 4-7; early-round slack absorbs the weave.
Uploads ride the Scalar DMA queue, Wp/bias the GpSimd queue, inputs
the sync queue; indirect downloads ride GpSimd, emitted at the round
where their CC lands to avoid head-of-line blocking the broadcast
chain.  expT is triple-buffered to break the ACT-vs-woven-PV WAR
chain.  NOTE: measured runs vary ~180-220us with activity-based PE
throttling (util limit 0.5 kicks in for tens of us per run).

Compute is bf16 on the TensorEngine with fp32 PSUM accumulation;
softmax is computed without max-subtraction (logits are bounded ~2.5
here) as exp(S^T) @ [V*e, e] with V stationary read through a flat
128-column window ([V_h | e | spill]) so Fast Weight Load stays on
(433ns -> 216ns per PV matmul).  The denominator lands in PSUM row 64,
is bounced to SBUF partition 0 (the approx-reciprocal DVE op misreads
PSUM at base partition 64) for a fast approximate reciprocal on
VectorE, broadcast across partitions on GpSimd, and multiplied in on
VectorE.  e = exp(-5*(1-mask)) folds the additive mask penalty exactly.
Output is stored bf16 (rel-err budget has >6x headroom).
"""

import numpy as np
import ml_dtypes

import concourse.bass as bass
import concourse.mybir as mybir
import concourse.tile as tile
from concourse import bacc
from concourse.bass import ds

N_CORES = 8
B, N, C = 4, 1024, 1024
H = 16
D = C // H  # 64
NQ = N // 2  # query rows per core: 512
P = 128
KC = C // P  # 8 contraction chunks
SCALE = D ** -0.5
PAIR_GROUPS = [[0, 1], [2, 3], [4, 5], [6, 7]]

F32 = mybir.dt.float32
BF16 = mybir.dt.bfloat16
I32 = mybir.dt.int32
AF = mybir.ActivationFunctionType


def build_nc():
    nc = bacc.Bacc(None, num_devices=N_CORES)

    xT = nc.declare_dram_parameter("xT", [C, N], BF16, isOutput=False)
    W_q = nc.declare_dram_parameter("W_q", [C, C], BF16, isOutput=False)
    W_k = nc.declare_dram_parameter("W_k", [C, C // 2], BF16, isOutput=False)
    W_v = nc.declare_dram_parameter("W_v", [C, C // 2], BF16, isOutput=False)
    W_p = nc.declare_dram_parameter("W_p", [C, C], BF16, isOutput=False)
    e_in = nc.declare_dram_parameter("e", [N], F32, isOutput=False)
    b_in = nc.declare_dram_parameter("b", [C], F32, isOutput=False)
    ridx_in = nc.declare_dram_parameter("ridx", [P, 12], I32, isOutput=False)
    out_ext = nc.declare_dram_parameter("out", [NQ, C], BF16, isOutput=True)

    with tile.TileContext(nc) as tc, (
        tc.tile_pool(name="acts", bufs=1)
    ) as apool, (
        tc.tile_pool(name="early", bufs=1)
    ) as early, (
        tc.tile_pool(name="work", bufs=2)
    ) as work, (
        tc.tile_pool(name="chain", bufs=2)
    ) as chain, (
        tc.tile_pool(name="dram", bufs=1, space="DRAM")
    ) as dram, (
        tc.tile_pool(name="ps_big", bufs=2, space="PSUM")
    ) as ps_big, (
        tc.tile_pool(name="ps_pv", bufs=2, space="PSUM")
    ) as ps_pv:
        e_s = apool.tile([P, KC], F32)
        nc.sync.dma_start(e_s[:], e_in.rearrange("(o p) -> p o", p=P))
        ridx_s = apool.tile([P, 12], I32)
        nc.sync.dma_start(ridx_s[:], ridx_in[:])
        Kst_r = apool.tile([P, 4096], BF16)
        V_r = apool.tile([P, KC, 512], BF16)

        # ---- PE warmup while the loads stream in --------------------
        warm = early.tile([P, 512], BF16, tag="warm")
        nc.vector.memset(warm[:], 0.0)
        wps = ps_big.tile([P, 1024], F32, tag="big")
        for i in range(6):
            nc.tensor.matmul(wps[:, 0:512], warm[:, 0:P], warm[:],
                             start=True, stop=True)

        # ---- static loads (chunked so matmuls can start early) ------
        xT_s = early.tile([P, KC, N], BF16)
        Wv_s = early.tile([P, KC, C // 2], BF16)
        Wk_s = early.tile([P, KC, C // 2], BF16)
        Wq_s = early.tile([P, KC, C], BF16)
        # All input loads ride the sync queue (the fastest ring) in
        # the order the compute needs them; the host orders xT columns
        # [own queries | rest] so the Q projection reads xT_s directly
        # and no separate xqT load is needed.  Wp/bias ride GpSimd.
        # xT is the long pole for round 0 (k0/q0 contract over all of
        # it): stripe it across the sync and scalar rings so it lands
        # in half the time; the scalar ring is otherwise idle until the
        # first upload (~30us).
        for kc in range(KC):
            eng = nc.sync if kc % 2 == 0 else nc.scalar
            eng.dma_start(xT_s[:, kc, :], xT[kc * P:(kc + 1) * P, :])
            nc.sync.dma_start(Wk_s[:, kc, :], W_k[kc * P:(kc + 1) * P, :])
        nc.sync.dma_start(Wq_s[:, :, 0:2 * P],
                          W_q.rearrange("(ko p) n -> p ko n", p=P)[:, :, 0:2 * P])
        for kc in range(KC):
            nc.sync.dma_start(Wv_s[:, kc, :], W_v[kc * P:(kc + 1) * P, :])
        nc.sync.dma_start(Wq_s[:, :, 2 * P:],
                          W_q.rearrange("(ko p) n -> p ko n", p=P)[:, :, 2 * P:])
        Wp_s = apool.tile([P, KC, C], BF16)
        bias_s = apool.tile([P, C], F32)
        nc.gpsimd.dma_start(Wp_s[:], W_p.rearrange("(ko p) n -> p ko n", p=P))
        nc.gpsimd.dma_start(bias_s[:], b_in[None, :].to_broadcast((P, C)))

        # V_s is flat so each head's PV stationary can be read as a
        # 128-column window [V_h | e | spill-into-next-head]: a 128-col
        # stationary keeps Fast Weight Load on.  The 63-col tail pad
        # keeps the last head's window in bounds; spill/pad columns
        # only feed PV output rows 65-127, which are never read.
        KT_s = apool.tile([P, KC, N], BF16)
        QT_s = apool.tile([P, KC, NQ], BF16)
        V_f = apool.tile([P, KC * H * (D + 1) + 63], BF16)
        V_s = V_f[:, 0:KC * H * (D + 1)].rearrange(
            "p (k h c) -> p k h c", h=H, c=D + 1)
        PT_s = apool.tile([P, KC, NQ], BF16)
        for mc in range(KC):
            nc.vector.tensor_copy(
                V_s[:, mc, :, D:D + 1],
                e_s[:, mc:mc + 1, None].to_broadcast((P, H, 1)),
            )

        def v_stat(kc, h):
            """128-col PV stationary window for head h, key chunk kc."""
            s = (kc * H + h) * (D + 1)
            return V_f[:, s:s + 128]

        # ---- DRAM bounce buffers + pair AllGathers ------------------
        # Three pipelined pair-gathers, ordered by when the remote half
        # is needed: KT fc0-1 (round 4), V (round 5's woven PV), KT
        # fc2-3 (rounds 6-7).  The first CC op can only execute once
        # the cross-core rendezvous (~30us start skew) resolves, so
        # small pipelined ops beat one big late one.  Remote halves are
        # fetched with indirect (index-tensor) DMAs — the per-core rank
        # offset lives in the host-provided ridx input, since runtime
        # registers are not supported in this environment.
        in_ka = dram.tile([P, 2, N], BF16)
        out_ka = dram.tile([2 * P, 2, N], BF16)
        in_v = dram.tile([KC, P, 512], BF16)
        out_v = dram.tile([2, KC, P, 512], BF16)
        in_kb = dram.tile([2, P, N], BF16)
        out_kb = dram.tile([2, 2, P, N], BF16)

        def gather(in_t, out_t):
            nc.gpsimd.collective_compute(
                "AllGather",
                mybir.AluOpType.bypass,
                replica_groups=PAIR_GROUPS,
                ins=[in_t[:].opt()],
                outs=[out_t[:].opt()],
            )

        def v_half(lo, accs):
            # One half of the V projection (key-chunks lo..lo+3),
            # kc-outer across four [P, 512] accumulators.  Half 1 uses
            # the big pool BEFORE round 0 (+~7us start); half 2 uses
            # the pv pool right after round 0's S^T — it only pins the
            # PV pool, which nothing touches until round 1's end, so
            # round 1's S^T is never blocked (the monolithic V pinned
            # all 8 banks and stalled it ~10us).
            for kc in range(KC):
                for m in range(4):
                    nc.tensor.matmul(
                        accs[m],
                        xT_s[:, kc, (lo + m) * P:(lo + m + 1) * P],
                        Wv_s[:, kc, :],
                        start=(kc == 0),
                        stop=(kc == KC - 1),
                    )
            for m in range(4):
                nc.vector.tensor_scalar_mul(
                    V_s[:, lo + m, 0:8, 0:D],
                    accs[m].rearrange("p (h d) -> p h d", d=D),
                    e_s[:, lo + m:lo + m + 1],
                )

        def v_upload():
            for kc in range(KC):
                nc.scalar.dma_start(
                    in_v[kc, :, :].rearrange("p (h d) -> p h d", d=D),
                    V_s[:, kc, 0:8, 0:D])

        def k_chunk(fc):
            """Own K^T feature chunk fc (virtual heads 2fc, 2fc+1)."""
            ps = ps_big.tile([P, 1024], F32, tag="big", name="ps_k")
            for nh in range(2):
                for kc in range(KC):
                    nc.tensor.matmul(
                        ps[:, nh * 512:(nh + 1) * 512],
                        Wk_s[:, kc, fc * P:(fc + 1) * P],
                        xT_s[:, kc, nh * 512:(nh + 1) * 512],
                        start=(kc == 0),
                        stop=(kc == KC - 1),
                    )
            nc.vector.tensor_copy(KT_s[:, fc, :], ps[:])

        def q_chunk(fc2):
            ps = ps_big.tile([P, 1024], F32, tag="big", name="ps_q")
            for half in range(2):
                fc = 2 * fc2 + half
                for kc in range(KC):
                    nc.tensor.matmul(
                        ps[:, half * 512:(half + 1) * 512],
                        Wq_s[:, kc, fc * P:(fc + 1) * P],
                        xT_s[:, kc, 0:NQ],
                        start=(kc == 0),
                        stop=(kc == KC - 1),
                    )
            nc.vector.tensor_copy(
                QT_s[:, 2 * fc2:2 * fc2 + 2, :],
                ps[:].rearrange("p (a b) -> p a b", a=2),
            )

        def st_exp(hp, expT, pv_hp=None, pv_expT=None, pv=None,
                   pv_points={1: [0], 3: [1]}):
            """S^T+exp tiles for pair hp.  When a previous pair's PV is
            supplied, its matmuls are emitted in two 8-matmul chunks
            between S^T groups: ScalarE keeps a 2-tile exp backlog to
            drain while TensorE runs the PV chunk.  S^T h0/h64 matmuls
            are emitted adjacently so the 64-row tiles can co-run."""
            for kcp in range(4):
                pss = [ps_big.tile([P, 1024], F32, tag="big",
                                   name=f"ps_st{h01}") for h01 in range(2)]
                for j in range(2):
                    kc = 2 * kcp + j
                    for h01 in range(2):
                        lo, hi = h01 * 64, h01 * 64 + 64
                        nc.tensor.matmul(
                            pss[h01][:, j * 512:(j + 1) * 512],
                            KT_s[lo:hi, hp, kc * P:(kc + 1) * P],
                            QT_s[lo:hi, hp, :],
                            start=True,
                            stop=True,
                        )
                for h01 in range(2):
                    nc.scalar.activation(
                        expT[h01][:, 2 * kcp:2 * kcp + 2, :],
                        pss[h01][:].rearrange("p (a b) -> p a b", a=2),
                        AF.Exp,
                        scale=SCALE,
                    )
                if pv is not None and kcp in pv_points:
                    for grp in pv_points[kcp]:
                        for h01 in range(2):
                            for kc in range(4 * grp, 4 * grp + 4):
                                nc.tensor.matmul(
                                    pv[:, h01, :],
                                    v_stat(kc, 2 * pv_hp + h01),
                                    pv_expT[h01][:, kc, :],
                                    start=(kc == 0),
                                    stop=(kc == KC - 1),
                                )

        def pv_norm(hp, expT, pv):
            # reciprocal_approx_fast's bitwise-NOT DVE op misreads PSUM at
            # base partition 64 (verified on hw) — bounce the denominator
            # row through SBUF partition 0 first.
            den = chain.tile([1, 2, NQ], F32, tag="den", bufs=1)
            rcr = chain.tile([1, 2, NQ], F32, tag="rcr", bufs=1)
            nc.vector.tensor_copy(den[0:1], pv[D:D + 1, :, :])
            nc.vector.reciprocal_approx_fast(rcr[0:1], den[0:1])
            for h01 in range(2):
                bcast = chain.tile([D, NQ], F32, tag=f"bcast{h01}",
                                   name=f"bcast{h01}")
                nc.gpsimd.partition_broadcast(bcast[:], rcr[0:1, h01, :])
                nc.vector.tensor_mul(
                    PT_s[h01 * D:(h01 + 1) * D, hp, :],
                    pv[0:D, h01, :],
                    bcast[:],
                )

        # ---- own K fc0-1, first Q chunk; CC1 ------------------------
        k_chunk(0)
        k_chunk(1)
        nc.scalar.dma_start(in_ka[:], KT_s[:, 0:2, :])
        gather(in_ka, out_ka)
        q_chunk(0)
        vb = [ps_big.tile([P, 1024], F32, tag="big", name=f"vh1{i}")
              for i in range(2)]
        v_half(0, [vb[m // 2][:, (m % 2) * 512:(m % 2 + 1) * 512]
                   for m in range(4)])

        # ---- software-pipelined attention rounds --------------------
        expTs = {}

        def new_expT(i):
            return [
                work.tile([P, KC, NQ], BF16, tag=f"exp{i % 3}_{h01}",
                          name=f"expT{h01}", bufs=1)
                for h01 in range(2)
            ]

        def proj_accum(ps, qs, fcs, start):
            for nn in range(2):
                for fc in fcs:
                    nc.tensor.matmul(
                        ps[:, nn * 512:(nn + 1) * 512],
                        PT_s[:, fc, qs * P:(qs + 1) * P],
                        Wp_s[:, fc, nn * 512:(nn + 1) * 512],
                        start=(start and fc == fcs[0]),
                        stop=(fc == KC - 1),
                    )

        def proj_store(ps, qs):
            o_sb = work.tile([P, 1024], BF16, tag="osb")
            nc.vector.tensor_add(o_sb[:], ps[:], bias_s[:])
            nc.sync.dma_start(out_ext[qs * P:(qs + 1) * P, :], o_sb[:])

        expTs[0] = new_expT(0)
        st_exp(0, expTs[0])
        vp = [ps_pv.tile([P, 2, NQ], F32, tag="pv", name=f"vh2{i}")
              for i in range(2)]
        v_half(4, [vp[m // 2][:, m % 2, :] for m in range(4)])
        v_upload()
        gather(in_v, out_v)
        for i in range(1, KC):
            expTs[i] = new_expT(i)
            pv = ps_pv.tile([P, 2, NQ], F32, tag="pv", name="pv")
            # Round 5's woven PV is the first to touch remote V; weave
            # it late in the round for extra exchange margin.
            pts = {3: [0, 1]} if i in (1, 5) else {1: [0], 3: [1]}
            st_exp(i, expTs[i], i - 1, expTs[i - 1], pv, pv_points=pts)
            pv_norm(i - 1, expTs[i - 1], pv)
            if i == 1:
                k_chunk(2)
                q_chunk(1)
                k_chunk(3)
                # CC3's upload + trigger ride behind round 1 so it
                # queues immediately after CC2 on the CC cores instead
                # of ~25us later (inter-op spacing gated rounds 6-7).
                for j in range(2):
                    nc.gpsimd.dma_start(in_kb[j, :, :], KT_s[:, 2 + j, :])
                gather(in_kb, out_kb)
            if i == 3:
                q_chunk(2)
            if i == 4:
                q_chunk(3)
            # Indirect remote-half downloads, emitted at the round where
            # their CC lands so they don't head-of-line-block the
            # GpSimd broadcast chain.  Indirect DMA destinations must be
            # contiguous (strided dsts mis-generate descriptors), so
            # they land in staging tiles and VectorE places them.
            if i == 2:
                nc.gpsimd.indirect_dma_start(
                    out=Kst_r[:, 0:2048],
                    out_offset=None,
                    in_=out_ka[:],
                    in_offset=bass.IndirectOffsetOnAxis(
                        ap=ridx_s[:, 0:1], axis=0),
                )
                nc.vector.tensor_copy(
                    KT_s[:, 4:6, :],
                    Kst_r[:, 0:2048].rearrange("p (a b) -> p a b", a=2))
            if i == 3:
                ovf = out_v[:].rearrange("r s p c -> (r s p) c")
                for kc in range(KC):
                    nc.gpsimd.indirect_dma_start(
                        out=V_r[:, kc, :],
                        out_offset=None,
                        in_=ovf,
                        in_offset=bass.IndirectOffsetOnAxis(
                            ap=ridx_s[:, 1 + kc:2 + kc], axis=0),
                    )
                for kc in range(KC):
                    nc.vector.tensor_copy(
                        V_s[:, kc, 8:16, 0:D],
                        V_r[:, kc, :].rearrange("p (h d) -> p h d", d=D))
            if i == 4:
                okf = out_kb[:].rearrange("r s p c -> (r s p) c")
                for j in range(2):
                    nc.gpsimd.indirect_dma_start(
                        out=Kst_r[:, 2048 + j * 1024:2048 + (j + 1) * 1024],
                        out_offset=None,
                        in_=okf,
                        in_offset=bass.IndirectOffsetOnAxis(
                            ap=ridx_s[:, 9 + j:10 + j], axis=0),
                    )
                nc.vector.tensor_copy(
                    KT_s[:, 6:8, :],
                    Kst_r[:, 2048:4096].rearrange("p (a b) -> p a b", a=2))

        # ---- output projection + bias -------------------------------
        # First three query chunks accumulate feature chunks 0..6 early
        # so TensorE stays busy while ScalarE drains the last pair's
        # exps; the fc=7 matmuls land after pv_norm(7) writes PT chunk 7.
        # pj2 takes the pv-ring buffer freed by pair 5's normalize, so
        # its prefix accumulation back-fills TensorE during round 7; pv7
        # then reuses pair 6's buffer (waits its normalize reads).
        ps_pj = {}
        ps_pj[2] = ps_pv.tile([P, 2, NQ], F32, tag="pv",
                              name="ps_pj2").rearrange("p a b -> p (a b)")
        proj_accum(ps_pj[2], 2, list(range(KC - 1)), start=True)
        pv7 = ps_pv.tile([P, 2, NQ], F32, tag="pv", name="pv")
        for h01 in range(2):
            for kc in range(KC):
                nc.tensor.matmul(
                    pv7[:, h01, :],
                    v_stat(kc, 2 * (KC - 1) + h01),
                    expTs[KC - 1][h01][:, kc, :],
                    start=(kc == 0),
                    stop=(kc == KC - 1),
                )
        for qs in range(2):
            ps_pj[qs] = ps_big.tile([P, 1024], F32, tag="big",
                                    name=f"ps_pj{qs}")
            proj_accum(ps_pj[qs], qs, list(range(KC - 1)), start=True)
        pv_norm(KC - 1, expTs[KC - 1], pv7)
        for qs in range(3):
            proj_accum(ps_pj[qs], qs, [KC - 1], start=False)
            proj_store(ps_pj[qs], qs)
        ps3 = ps_big.tile([P, 1024], F32, tag="big", name="ps_pj3")
        proj_accum(ps3, 3, list(range(KC)), start=True)
        proj_store(ps3, 3)

    nc.finalize()
    return nc


def make_in_maps(x, mask, W_qkv, W_proj, b_proj):
    bf = ml_dtypes.bfloat16
    x = np.asarray(x, np.float32)
    mask = np.asarray(mask, np.float32)
    W_qkv = np.asarray(W_qkv, np.float32)
    W_proj = np.asarray(W_proj, np.float32)
    b_proj = np.asarray(b_proj, np.float32)

    Wq_full = W_qkv[:, 0:C]
    Wk_full = W_qkv[:, C:2 * C]
    Wv_full = W_qkv[:, 2 * C:3 * C]
    e_all = np.exp(-5.0 * (1.0 - mask)).astype(np.float32)  # [B, N]

    def ridx_for(g):
        """Row indices of the peer rank's half in each gather output."""
        p = np.arange(P, dtype=np.int32)
        r = 1 - g
        cols = [r * P + p]                                  # out_ka [2P,...]
        cols += [r * 1024 + s * P + p for s in range(8)]    # out_v flat
        cols += [r * 256 + j * P + p for j in range(2)]     # out_kb flat
        cols += [np.zeros(P, np.int32)]
        return np.stack(cols, axis=1).astype(np.int32)

    in_maps = []
    for c in range(N_CORES):
        b, g = divmod(c, 2)
        own = slice(g * 512, (g + 1) * 512)
        rem = slice((1 - g) * 512, (2 - g) * 512)
        xq = x[b, g * NQ:(g + 1) * NQ, :]
        xo = x[b, (1 - g) * NQ:(2 - g) * NQ, :]
        xT = np.ascontiguousarray(
            np.concatenate([xq, xo], axis=0).T).astype(bf)
        W_q = np.ascontiguousarray(
            np.concatenate([Wq_full[:, own], Wq_full[:, rem]], axis=1)
        ).astype(bf)
        W_k = np.ascontiguousarray(Wk_full[:, own]).astype(bf)
        W_v = np.ascontiguousarray(Wv_full[:, own]).astype(bf)
        W_p = np.ascontiguousarray(
            np.concatenate([W_proj[own, :], W_proj[rem, :]], axis=0)
        ).astype(bf)
        in_maps.append({
            "xT": xT, "W_q": W_q, "W_k": W_k, "W_v": W_v,
            "W_p": W_p,
            "e": np.ascontiguousarray(
                np.concatenate([e_all[b, g * NQ:(g + 1) * NQ],
                                e_all[b, (1 - g) * NQ:(2 - g) * NQ]])),
            "b": b_proj,
            "ridx": ridx_for(g),
        })
    return in_maps


def assemble_output(results):
    out = np.zeros((B, N, C), np.float32)
    for c in range(N_CORES):
        b, g = divmod(c, 2)
        out[b, g * NQ:(g + 1) * NQ, :] = np.asarray(
            results[c]["out"], np.float32)
    return out


def kernel(x, mask, W_qkv, W_proj, b_proj):
    from concourse.bass_utils import run_bass_kernel_spmd

    nc = build_nc()
    in_maps = make_in_maps(x, mask, W_qkv, W_proj, b_proj)
    res = run_bass_kernel_spmd(nc, in_maps, core_ids=list(range(N_CORES)))
    return assemble_output(res.results)


# revision 61
# speedup vs baseline: 1.0086x; 1.0086x over previous
"""Multi-head attention (B=4, N=1024, C=1024, H=16) on 8 TRN2 NeuronCores.

Sharding: core c handles batch b = c//2 and query-row half g = c%2.
Data parallel over B; within a batch pair, K and V are tensor parallel
over heads.  All per-head state lives in VIRTUAL head order — the host
permutes W_q / W_k / W_p per core so that each core's own 8 heads are
virtual heads 0-7 — which makes the kernel program identical on every
core: attention rounds process own head-pairs first, so the K/V
exchange (three pipelined pair AllGathers through DRAM: KT fc0-1, V,
KT fc2-3) only has to land by rounds 4/5/6.  The peer's half is
fetched with indirect (index-tensor) DMAs into contiguous staging and
placed by VectorE — the per-core rank offset lives in the host-built
ridx input, because runtime registers (value_load / dynamic AP
offsets) crash this runtime, and strided indirect-DMA destinations
corrupt.  The host also orders each core's xT key-columns [own queries
| rest], so Q projects straight out of xT_s and no xqT load exists.

Schedule: rounds start as soon as k0,k1,q0 land (~30us); the V
projection (kc-outer across 8 PSUM accumulators carved from the
big+pv pools), remaining k/q chunks, the output-projection prefix
accumulations, and the woven PV matmuls all backfill TensorE inside
the ScalarE(exp)-bound rounds.  The first CC op only executes ~19us
after the ~35us start-skew barrier resolves, so the CC chain (not
compute) gates rounds 4-7; early-round slack absorbs the weave.
Uploads ride the Scalar DMA queue, Wp/bias the GpSimd queue, inputs
the sync queue; indirect downloads ride GpSimd, emitted at the round
where their CC lands to avoid head-of-line blocking the broadcast
chain.  expT is triple-buffered to break the ACT-vs-woven-PV WAR
chain.  NOTE: measured runs vary ~180-220us with activity-based PE
throttling (util limit 0.5 kicks in for tens of us per run).

Compute is bf16 on the TensorEngine with fp32 PSUM accumulation;
softmax is computed without max-subtraction (logits are bounded ~2.5
here) as exp(S^T) @ [V*e, e] with V stationary read through a flat
128-column window ([V_h | e | spill]) so Fast Weight Load stays on
(433ns -> 216ns per PV matmul).  The denominator lands in PSUM row 64,
is bounced to SBUF partition 0 (the approx-reciprocal DVE op misreads
PSUM at base partition 64) for a fast approximate reciprocal on
VectorE, broadcast across partitions on GpSimd, and multiplied in on
VectorE.  e = exp(-5*(1-mask)) folds the additive mask penalty exactly.
Output is stored bf16 (rel-err budget has >6x headroom).
"""

import numpy as np
import ml_dtypes

import concourse.bass as bass
import concourse.mybir as mybir
import concourse.tile as tile
from concourse import bacc
from concourse.bass import ds

N_CORES = 8
B, N, C = 4, 1024, 1024
H = 16
D = C // H  # 64
NQ = N // 2  # query rows per core: 512
P = 128
KC = C // P  # 8 contraction chunks
SCALE = D ** -0.5
PAIR_GROUPS = [[0, 1], [2, 3], [4, 5], [6, 7]]

F32 = mybir.dt.float32
BF16 = mybir.dt.bfloat16
I32 = mybir.dt.int32
AF = mybir.ActivationFunctionType


def build_nc():
    nc = bacc.Bacc(None, num_devices=N_CORES)

    xT = nc.declare_dram_parameter("xT", [C, N], BF16, isOutput=False)
    W_q = nc.declare_dram_parameter("W_q", [C, C], BF16, isOutput=False)
    W_k = nc.declare_dram_parameter("W_k", [C, C // 2], BF16, isOutput=False)
    W_v = nc.declare_dram_parameter("W_v", [C, C // 2], BF16, isOutput=False)
    W_p = nc.declare_dram_parameter("W_p", [C, C], BF16, isOutput=False)
    e_in = nc.declare_dram_parameter("e", [N], F32, isOutput=False)
    b_in = nc.declare_dram_parameter("b", [C], F32, isOutput=False)
    ridx_in = nc.declare_dram_parameter("ridx", [P, 12], I32, isOutput=False)
    out_ext = nc.declare_dram_parameter("out", [NQ, C], BF16, isOutput=True)

    with tile.TileContext(nc) as tc, (
        tc.tile_pool(name="acts", bufs=1)
    ) as apool, (
        tc.tile_pool(name="early", bufs=1)
    ) as early, (
        tc.tile_pool(name="work", bufs=2)
    ) as work, (
        tc.tile_pool(name="chain", bufs=2)
    ) as chain, (
        tc.tile_pool(name="dram", bufs=1, space="DRAM")
    ) as dram, (
        tc.tile_pool(name="ps_big", bufs=2, space="PSUM")
    ) as ps_big, (
        tc.tile_pool(name="ps_pv", bufs=2, space="PSUM")
    ) as ps_pv:
        e_s = apool.tile([P, KC], F32)
        nc.sync.dma_start(e_s[:], e_in.rearrange("(o p) -> p o", p=P))
        ridx_s = apool.tile([P, 12], I32)
        nc.sync.dma_start(ridx_s[:], ridx_in[:])
        Kst_r = apool.tile([P, 4096], BF16)
        V_r = apool.tile([P, KC, 512], BF16)

        # ---- PE warmup while the loads stream in --------------------
        warm = early.tile([P, 512], BF16, tag="warm")
        nc.vector.memset(warm[:], 0.0)
        wps = ps_big.tile([P, 1024], F32, tag="big")
        for i in range(6):
            nc.tensor.matmul(wps[:, 0:512], warm[:, 0:P], warm[:],
                             start=True, stop=True)

        # ---- static loads (chunked so matmuls can start early) ------
        xT_s = early.tile([P, KC, N], BF16)
        Wv_s = early.tile([P, KC, C // 2], BF16)
        Wk_s = early.tile([P, KC, C // 2], BF16)
        Wq_s = early.tile([P, KC, C], BF16)
        # All input loads ride the sync queue (the fastest ring) in
        # the order the compute needs them; the host orders xT columns
        # [own queries | rest] so the Q projection reads xT_s directly
        # and no separate xqT load is needed.  Wp/bias ride GpSimd.
        # xT is the long pole for round 0 (k0/q0 contract over all of
        # it): stripe it across the sync and scalar rings so it lands
        # in half the time; the scalar ring is otherwise idle until the
        # first upload (~30us).
        for kc in range(KC):
            eng = nc.sync if kc % 2 == 0 else nc.scalar
            eng.dma_start(xT_s[:, kc, :], xT[kc * P:(kc + 1) * P, :])
            nc.sync.dma_start(Wk_s[:, kc, :], W_k[kc * P:(kc + 1) * P, :])
        nc.sync.dma_start(Wq_s[:, :, 0:2 * P],
                          W_q.rearrange("(ko p) n -> p ko n", p=P)[:, :, 0:2 * P])
        for kc in range(KC):
            nc.sync.dma_start(Wv_s[:, kc, :], W_v[kc * P:(kc + 1) * P, :])
        nc.sync.dma_start(Wq_s[:, :, 2 * P:],
                          W_q.rearrange("(ko p) n -> p ko n", p=P)[:, :, 2 * P:])
        Wp_s = apool.tile([P, KC, C], BF16)
        bias_s = apool.tile([P, C], F32)
        nc.gpsimd.dma_start(Wp_s[:], W_p.rearrange("(ko p) n -> p ko n", p=P))
        nc.gpsimd.dma_start(bias_s[:], b_in[None, :].to_broadcast((P, C)))

        # V_s is flat so each head's PV stationary can be read as a
        # 128-column window [V_h | e | spill-into-next-head]: a 128-col
        # stationary keeps Fast Weight Load on.  The 63-col tail pad
        # keeps the last head's window in bounds; spill/pad columns
        # only feed PV output rows 65-127, which are never read.
        KT_s = apool.tile([P, KC, N], BF16)
        QT_s = apool.tile([P, KC, NQ], BF16)
        V_f = apool.tile([P, KC * H * (D + 1) + 63], BF16)
        V_s = V_f[:, 0:KC * H * (D + 1)].rearrange(
            "p (k h c) -> p k h c", h=H, c=D + 1)
        PT_s = apool.tile([P, KC, NQ], BF16)
        for mc in range(KC):
            nc.vector.tensor_copy(
                V_s[:, mc, :, D:D + 1],
                e_s[:, mc:mc + 1, None].to_broadcast((P, H, 1)),
            )

        def v_stat(kc, h):
            """128-col PV stationary window for head h, key chunk kc."""
            s = (kc * H + h) * (D + 1)
            return V_f[:, s:s + 128]

        # ---- DRAM bounce buffers + pair AllGathers ------------------
        # Three pipelined pair-gathers, ordered by when the remote half
        # is needed: KT fc0-1 (round 4), V (round 5's woven PV), KT
        # fc2-3 (rounds 6-7).  The first CC op can only execute once
        # the cross-core rendezvous (~30us start skew) resolves, so
        # small pipelined ops beat one big late one.  Remote halves are
        # fetched with indirect (index-tensor) DMAs — the per-core rank
        # offset lives in the host-provided ridx input, since runtime
        # registers are not supported in this environment.
        in_ka = dram.tile([P, 2, N], BF16)
        out_ka = dram.tile([2 * P, 2, N], BF16)
        in_v = dram.tile([KC, P, 512], BF16)
        out_v = dram.tile([2, KC, P, 512], BF16)
        in_kb = dram.tile([2, P, N], BF16)
        out_kb = dram.tile([2, 2, P, N], BF16)

        def gather(in_t, out_t):
            nc.gpsimd.collective_compute(
                "AllGather",
                mybir.AluOpType.bypass,
                replica_groups=PAIR_GROUPS,
                ins=[in_t[:].opt()],
                outs=[out_t[:].opt()],
            )

        def v_half(lo, accs):
            # One half of the V projection (key-chunks lo..lo+3),
            # kc-outer across four [P, 512] accumulators.  Half 1 uses
            # the big pool BEFORE round 0 (+~7us start); half 2 uses
            # the pv pool right after round 0's S^T — it only pins the
            # PV pool, which nothing touches until round 1's end, so
            # round 1's S^T is never blocked (the monolithic V pinned
            # all 8 banks and stalled it ~10us).
            for kc in range(KC):
                for m in range(4):
                    nc.tensor.matmul(
                        accs[m],
                        xT_s[:, kc, (lo + m) * P:(lo + m + 1) * P],
                        Wv_s[:, kc, :],
                        start=(kc == 0),
                        stop=(kc == KC - 1),
                    )
            for m in range(4):
                nc.vector.tensor_scalar_mul(
                    V_s[:, lo + m, 0:8, 0:D],
                    accs[m].rearrange("p (h d) -> p h d", d=D),
                    e_s[:, lo + m:lo + m + 1],
                )

        def v_upload():
            for kc in range(KC):
                nc.scalar.dma_start(
                    in_v[kc, :, :].rearrange("p (h d) -> p h d", d=D),
                    V_s[:, kc, 0:8, 0:D])

        def k_chunk(fc):
            """Own K^T feature chunk fc (virtual heads 2fc, 2fc+1)."""
            ps = ps_big.tile([P, 1024], F32, tag="big", name="ps_k")
            for nh in range(2):
                for kc in range(KC):
                    nc.tensor.matmul(
                        ps[:, nh * 512:(nh + 1) * 512],
                        Wk_s[:, kc, fc * P:(fc + 1) * P],
                        xT_s[:, kc, nh * 512:(nh + 1) * 512],
                        start=(kc == 0),
                        stop=(kc == KC - 1),
                    )
            nc.vector.tensor_copy(KT_s[:, fc, :], ps[:])

        def q_chunk(fc2):
            ps = ps_big.tile([P, 1024], F32, tag="big", name="ps_q")
            for half in range(2):
                fc = 2 * fc2 + half
                for kc in range(KC):
                    nc.tensor.matmul(
                        ps[:, half * 512:(half + 1) * 512],
                        Wq_s[:, kc, fc * P:(fc + 1) * P],
                        xT_s[:, kc, 0:NQ],
                        start=(kc == 0),
                        stop=(kc == KC - 1),
                    )
            nc.vector.tensor_copy(
                QT_s[:, 2 * fc2:2 * fc2 + 2, :],
                ps[:].rearrange("p (a b) -> p a b", a=2),
            )

        def st_exp(hp, expT, pv_hp=None, pv_expT=None, pv=None,
                   pv_points={1: [0], 3: [1]}):
            """S^T+exp tiles for pair hp.  When a previous pair's PV is
            supplied, its matmuls are emitted in two 8-matmul chunks
            between S^T groups: ScalarE keeps a 2-tile exp backlog to
            drain while TensorE runs the PV chunk.  S^T h0/h64 matmuls
            are emitted adjacently so the 64-row tiles can co-run."""
            for kcp in range(4):
                pss = [ps_big.tile([P, 1024], F32, tag="big",
                                   name=f"ps_st{h01}") for h01 in range(2)]
                for j in range(2):
                    kc = 2 * kcp + j
                    for h01 in range(2):
                        lo, hi = h01 * 64, h01 * 64 + 64
                        nc.tensor.matmul(
                            pss[h01][:, j * 512:(j + 1) * 512],
                            KT_s[lo:hi, hp, kc * P:(kc + 1) * P],
                            QT_s[lo:hi, hp, :],
                            start=True,
                            stop=True,
                        )
                for h01 in range(2):
                    nc.scalar.activation(
                        expT[h01][:, 2 * kcp:2 * kcp + 2, :],
                        pss[h01][:].rearrange("p (a b) -> p a b", a=2),
                        AF.Exp,
                        scale=SCALE,
                    )
                if pv is not None and kcp in pv_points:
                    for grp in pv_points[kcp]:
                        for h01 in range(2):
                            for kc in range(4 * grp, 4 * grp + 4):
                                nc.tensor.matmul(
                                    pv[:, h01, :],
                                    v_stat(kc, 2 * pv_hp + h01),
                                    pv_expT[h01][:, kc, :],
                                    start=(kc == 0),
                                    stop=(kc == KC - 1),
                                )

        def pv_norm(hp, expT, pv):
            # reciprocal_approx_fast's bitwise-NOT DVE op misreads PSUM at
            # base partition 64 (verified on hw) — bounce the denominator
            # row through SBUF partition 0 first.
            den = chain.tile([1, 2, NQ], F32, tag="den", bufs=1)
            rcr = chain.tile([1, 2, NQ], F32, tag="rcr", bufs=1)
            nc.vector.tensor_copy(den[0:1], pv[D:D + 1, :, :])
            nc.vector.reciprocal_approx_fast(rcr[0:1], den[0:1])
            for h01 in range(2):
                bcast = chain.tile([D, NQ], F32, tag=f"bcast{h01}",
                                   name=f"bcast{h01}")
                nc.gpsimd.partition_broadcast(bcast[:], rcr[0:1, h01, :])
                nc.vector.tensor_mul(
                    PT_s[h01 * D:(h01 + 1) * D, hp, :],
                    pv[0:D, h01, :],
                    bcast[:],
                )

        # ---- own K fc0-1, first Q chunk; CC1 ------------------------
        k_chunk(0)
        q_chunk(0)
        k_chunk(1)
        nc.scalar.dma_start(in_ka[:], KT_s[:, 0:2, :])
        gather(in_ka, out_ka)
        vb = [ps_big.tile([P, 1024], F32, tag="big", name=f"vh1{i}")
              for i in range(2)]
        v_half(0, [vb[m // 2][:, (m % 2) * 512:(m % 2 + 1) * 512]
                   for m in range(4)])

        # ---- software-pipelined attention rounds --------------------
        expTs = {}

        def new_expT(i):
            return [
                work.tile([P, KC, NQ], BF16, tag=f"exp{i % 3}_{h01}",
                          name=f"expT{h01}", bufs=1)
                for h01 in range(2)
            ]

        def proj_accum(ps, qs, fcs, start):
            for nn in range(2):
                for fc in fcs:
                    nc.tensor.matmul(
                        ps[:, nn * 512:(nn + 1) * 512],
                        PT_s[:, fc, qs * P:(qs + 1) * P],
                        Wp_s[:, fc, nn * 512:(nn + 1) * 512],
                        start=(start and fc == fcs[0]),
                        stop=(fc == KC - 1),
                    )

        def proj_store(ps, qs):
            o_sb = work.tile([P, 1024], BF16, tag="osb")
            nc.vector.tensor_add(o_sb[:], ps[:], bias_s[:])
            nc.sync.dma_start(out_ext[qs * P:(qs + 1) * P, :], o_sb[:])

        expTs[0] = new_expT(0)
        st_exp(0, expTs[0])
        vp = [ps_pv.tile([P, 2, NQ], F32, tag="pv", name=f"vh2{i}")
              for i in range(2)]
        v_half(4, [vp[m // 2][:, m % 2, :] for m in range(4)])
        v_upload()
        gather(in_v, out_v)
        for i in range(1, KC):
            expTs[i] = new_expT(i)
            pv = ps_pv.tile([P, 2, NQ], F32, tag="pv", name="pv")
            # Round 5's woven PV is the first to touch remote V; weave
            # it late in the round for extra exchange margin.
            pts = {3: [0, 1]} if i == 5 else {1: [0], 3: [1]}
            st_exp(i, expTs[i], i - 1, expTs[i - 1], pv, pv_points=pts)
            pv_norm(i - 1, expTs[i - 1], pv)
            if i == 1:
                k_chunk(2)
                q_chunk(1)
                k_chunk(3)
                # CC3's upload + trigger ride behind round 1 so it
                # queues immediately after CC2 on the CC cores instead
                # of ~25us later (inter-op spacing gated rounds 6-7).
                for j in range(2):
                    nc.gpsimd.dma_start(in_kb[j, :, :], KT_s[:, 2 + j, :])
                gather(in_kb, out_kb)
            if i == 3:
                q_chunk(2)
            if i == 4:
                q_chunk(3)
            # Indirect remote-half downloads, emitted at the round where
            # their CC lands so they don't head-of-line-block the
            # GpSimd broadcast chain.  Indirect DMA destinations must be
            # contiguous (strided dsts mis-generate descriptors), so
            # they land in staging tiles and VectorE places them.
            if i == 2:
                nc.gpsimd.indirect_dma_start(
                    out=Kst_r[:, 0:2048],
                    out_offset=None,
                    in_=out_ka[:],
                    in_offset=bass.IndirectOffsetOnAxis(
                        ap=ridx_s[:, 0:1], axis=0),
                )
                nc.vector.tensor_copy(
                    KT_s[:, 4:6, :],
                    Kst_r[:, 0:2048].rearrange("p (a b) -> p a b", a=2))
            if i == 3:
                ovf = out_v[:].rearrange("r s p c -> (r s p) c")
                for kc in range(KC):
                    nc.gpsimd.indirect_dma_start(
                        out=V_r[:, kc, :],
                        out_offset=None,
                        in_=ovf,
                        in_offset=bass.IndirectOffsetOnAxis(
                            ap=ridx_s[:, 1 + kc:2 + kc], axis=0),
                    )
                for kc in range(KC):
                    nc.vector.tensor_copy(
                        V_s[:, kc, 8:16, 0:D],
                        V_r[:, kc, :].rearrange("p (h d) -> p h d", d=D))
            if i == 4:
                okf = out_kb[:].rearrange("r s p c -> (r s p) c")
                for j in range(2):
                    nc.gpsimd.indirect_dma_start(
                        out=Kst_r[:, 2048 + j * 1024:2048 + (j + 1) * 1024],
                        out_offset=None,
                        in_=okf,
                        in_offset=bass.IndirectOffsetOnAxis(
                            ap=ridx_s[:, 9 + j:10 + j], axis=0),
                    )
                nc.vector.tensor_copy(
                    KT_s[:, 6:8, :],
                    Kst_r[:, 2048:4096].rearrange("p (a b) -> p a b", a=2))

        # ---- output projection + bias -------------------------------
        # First three query chunks accumulate feature chunks 0..6 early
        # so TensorE stays busy while ScalarE drains the last pair's
        # exps; the fc=7 matmuls land after pv_norm(7) writes PT chunk 7.
        # pj2 takes the pv-ring buffer freed by pair 5's normalize, so
        # its prefix accumulation back-fills TensorE during round 7; pv7
        # then reuses pair 6's buffer (waits its normalize reads).
        ps_pj = {}
        ps_pj[2] = ps_pv.tile([P, 2, NQ], F32, tag="pv",
                              name="ps_pj2").rearrange("p a b -> p (a b)")
        proj_accum(ps_pj[2], 2, list(range(KC - 1)), start=True)
        pv7 = ps_pv.tile([P, 2, NQ], F32, tag="pv", name="pv")
        for h01 in range(2):
            for kc in range(KC):
                nc.tensor.matmul(
                    pv7[:, h01, :],
                    v_stat(kc, 2 * (KC - 1) + h01),
                    expTs[KC - 1][h01][:, kc, :],
                    start=(kc == 0),
                    stop=(kc == KC - 1),
                )
        for qs in range(2):
            ps_pj[qs] = ps_big.tile([P, 1024], F32, tag="big",
                                    name=f"ps_pj{qs}")
            proj_accum(ps_pj[qs], qs, list(range(KC - 1)), start=True)
        pv_norm(KC - 1, expTs[KC - 1], pv7)
        for qs in range(3):
            proj_accum(ps_pj[qs], qs, [KC - 1], start=False)
            proj_store(ps_pj[qs], qs)
        ps3 = ps_big.tile([P, 1024], F32, tag="big", name="ps_pj3")
        proj_accum(ps3, 3, list(range(KC)), start=True)
        proj_store(ps3, 3)

    nc.finalize()
    return nc


def make_in_maps(x, mask, W_qkv, W_proj, b_proj):
    bf = ml_dtypes.bfloat16
    x = np.asarray(x, np.float32)
    mask = np.asarray(mask, np.float32)
    W_qkv = np.asarray(W_qkv, np.float32)
    W_proj = np.asarray(W_proj, np.float32)
    b_proj = np.asarray(b_proj, np.float32)

    Wq_full = W_qkv[:, 0:C]
    Wk_full = W_qkv[:, C:2 * C]
    Wv_full = W_qkv[:, 2 * C:3 * C]
    e_all = np.exp(-5.0 * (1.0 - mask)).astype(np.float32)  # [B, N]

    def ridx_for(g):
        """Row indices of the peer rank's half in each gather output."""
        p = np.arange(P, dtype=np.int32)
        r = 1 - g
        cols = [r * P + p]                                  # out_ka [2P,...]
        cols += [r * 1024 + s * P + p for s in range(8)]    # out_v flat
        cols += [r * 256 + j * P + p for j in range(2)]     # out_kb flat
        cols += [np.zeros(P, np.int32)]
        return np.stack(cols, axis=1).astype(np.int32)

    in_maps = []
    for c in range(N_CORES):
        b, g = divmod(c, 2)
        own = slice(g * 512, (g + 1) * 512)
        rem = slice((1 - g) * 512, (2 - g) * 512)
        xq = x[b, g * NQ:(g + 1) * NQ, :]
        xo = x[b, (1 - g) * NQ:(2 - g) * NQ, :]
        xT = np.ascontiguousarray(
            np.concatenate([xq, xo], axis=0).T).astype(bf)
        W_q = np.ascontiguousarray(
            np.concatenate([Wq_full[:, own], Wq_full[:, rem]], axis=1)
        ).astype(bf)
        W_k = np.ascontiguousarray(Wk_full[:, own]).astype(bf)
        W_v = np.ascontiguousarray(Wv_full[:, own]).astype(bf)
        W_p = np.ascontiguousarray(
            np.concatenate([W_proj[own, :], W_proj[rem, :]], axis=0)
        ).astype(bf)
        in_maps.append({
            "xT": xT, "W_q": W_q, "W_k": W_k, "W_v": W_v,
            "W_p": W_p,
            "e": np.ascontiguousarray(
                np.concatenate([e_all[b, g * NQ:(g + 1) * NQ],
                                e_all[b, (1 - g) * NQ:(2 - g) * NQ]])),
            "b": b_proj,
            "ridx": ridx_for(g),
        })
    return in_maps


def assemble_output(results):
    out = np.zeros((B, N, C), np.float32)
    for c in range(N_CORES):
        b, g = divmod(c, 2)
        out[b, g * NQ:(g + 1) * NQ, :] = np.asarray(
            results[c]["out"], np.float32)
    return out


def kernel(x, mask, W_qkv, W_proj, b_proj):
    from concourse.bass_utils import run_bass_kernel_spmd

    nc = build_nc()
    in_maps = make_in_maps(x, mask, W_qkv, W_proj, b_proj)
    res = run_bass_kernel_spmd(nc, in_maps, core_ids=list(range(N_CORES)))
    return assemble_output(res.results)
